# revision 53
# baseline (speedup 1.0000x reference)
"""CosineAttention Trainium2 Bass kernel (fp8 DoubleRow version).

Computes, per batch element b (one NeuronCore each, 8 cores total):
    proj   = x[b] @ W                      # [S, D]
    normed = proj / max(||proj||_2, eps)   # L2 normalize rows
    sim    = normed @ normed.T             # [S, S]
    out[b] = mean_s sigmoid(sim[s, t])     # [1, S]

Key design points (vs the fp32r/bf16 baseline, ~650us -> ~345us/rep):
  - Both big matmuls run in fp8-e4m3 with MatmulPerfMode.DoubleRow
    (K=256 per matmul): proj contracts w8 x xt pairs, sim contracts
    normed8 pairs.  Operands live in [128, NK, cols] k-major tiles so a
    [:, 2t:2t+2, cols] slice is DR-ready (weight AP pair-step must be
    a multiple of 16 -- s3_lw_dual_fp8_restrictions).
  - x tiles are DMA'd fp32 (alternating the SP and Activation HWDGE
    queues), cast to bf16, transposed on the PE with a bf16 identity
    (1 cycle/row), and evicted PSUM->SBUF as fp8 in one [128, NK, 128]
    strided copy.  Elementwise ops use nc.any so the tile scheduler
    load-balances ACT/DVE.
  - Norms fold into the per-j pipeline: ACT Square (PSUM) -> ones-matmul
    n2 -> short per-j chain (ACT Sqrt + DVE max/reciprocal) -> broadcast
    matmul -> normed8, all overlapped under the next block's matmuls.
    (A batched-once chain or any serial per-j chain on its own tail was
    worth ~50-100us.)  The x16 fp8 scaling is folded into the broadcast
    matmul (ones_row = 16).
  - sim tiles are computed in i-row groups of G=2 j-blocks so ONE ACT
    Sigmoid (scale=1/256 undoes the fp8 scaling) covers [128, G*512]
    with a single accum_out row-sum; symmetric column-sums use fp8
    DoubleRow ones-matmuls over scr (i, i+1) pair tiles.
  - Only the upper super-block triangle of sim is computed (j >= i//4);
    lower-triangle contributions come from column sums by symmetry.
"""

from contextlib import ExitStack

import numpy as np

import concourse.bacc as bacc
import concourse.mybir as mybir
import concourse.tile as tile
from concourse.masks import make_identity

FP32 = mybir.dt.float32
FP32R = mybir.dt.float32r
BF16 = mybir.dt.bfloat16
FP8 = mybir.dt.float8e4
FP8E5 = mybir.dt.float8e5
AF = mybir.ActivationFunctionType
ALU = mybir.AluOpType
AX = mybir.AxisListType
DR = mybir.MatmulPerfMode.DoubleRow

B = 8
S = 4096
D = 1024
EPS = 1e-12
N_CORES = 8
FP8_SCALE = 16.0  # normed rows scaled x16 into e4m3 sweet spot


def emit(ctx, tc, out_ap, x_ap, w_ap, s_total=S, repeats=1, phases="full",
         barrier=True):
    nc = tc.nc
    SB = 512                # s-block (matmul N)
    NB = s_total // SB      # number of s blocks
    NK = D // 128           # 128-row contraction tiles
    NP = NK // 2            # DoubleRow k-pair count
    NST = s_total // 128    # 128-row s tiles
    STPB = SB // 128        # s tiles per block

    const_pool = ctx.enter_context(tc.tile_pool(name="const", bufs=1))
    wpool = ctx.enter_context(tc.tile_pool(name="wpool", bufs=1))
    big_pool = ctx.enter_context(tc.tile_pool(name="big", bufs=1))
    xraw_pool = ctx.enter_context(tc.tile_pool(name="xraw", bufs=5))
    xbf_pool = ctx.enter_context(tc.tile_pool(name="xbf", bufs=4))
    xt_pool = ctx.enter_context(tc.tile_pool(name="xt", bufs=3))
    sq_pool = ctx.enter_context(tc.tile_pool(name="sq", bufs=4))
    psb_pool = ctx.enter_context(tc.tile_pool(name="psb", bufs=3))
    small_pool = ctx.enter_context(tc.tile_pool(name="small", bufs=3))
    bcs_pool = ctx.enter_context(tc.tile_pool(name="bcs", bufs=3))
    scr_pool = ctx.enter_context(tc.tile_pool(name="scr", bufs=4))
    acc_pool = ctx.enter_context(tc.tile_pool(name="acc", bufs=1))

    ident = const_pool.tile([128, 128], BF16, name="ident")
    make_identity(nc, ident)
    ident1 = const_pool.tile([1, 1], FP32, name="ident1")
    nc.vector.memset(ident1, 1.0)
    ones_colb = const_pool.tile([128, 1], BF16, name="ones_colb")
    nc.vector.memset(ones_colb, 1.0)
    # [128, 2, 16] so the DR pair-slice [:, :, 0:1] has step 16
    # (s3_lw_dual_fp8_restrictions: weight AP step % 16 == 0).
    ones8_t = const_pool.tile([128, 2, 16], FP8, name="ones8")
    nc.vector.memset(ones8_t, 1.0)
    ones8 = ones8_t[:, :, 0:1]

    ones_row_f = const_pool.tile([1, 128], FP32, name="ones_row_f")
    nc.vector.memset(ones_row_f, FP8_SCALE)
    ones_row = const_pool.tile([1, 128], FP32R, name="ones_row")
    nc.vector.tensor_copy(out=ones_row, in_=ones_row_f)
    out_all = const_pool.tile([128, NST], FP32, name="out_all")
    out_fin = const_pool.tile([128, NST], FP32, name="out_fin")
    sym_all = const_pool.tile([128, NST], FP32, name="sym_all")

    # W in fp8, k-major: w8[p, k, e] = W[k*128+p, e]
    w8 = wpool.tile([128, NK, D], FP8, name="w8")
    for k in range(NK):
        wf = xraw_pool.tile([128, D], FP32, name="wf", tag="wf")
        nc.sync.dma_start(out=wf, in_=w_ap[k * 128:(k + 1) * 128, :])
        nc.vector.tensor_copy(out=w8[:, k, :], in_=wf)

    # normalized proj in fp8 (k-major, x16)
    normed8 = big_pool.tile([128, NK, s_total], FP8, name="normed8")
    if phases == "p2":
        # p2-only build skips phase 1; init so tiles are allocated.
        nc.vector.memset(normed8, 0.01)
    rn_r = const_pool.tile([1, s_total], FP32R, name="rn_r")

    p1_sub = {"dma": 1, "tr": 2, "proj": 3, "p1a": 3.5}.get(phases, 4)
    run_p1 = phases in ("full", "p1", "dma", "tr", "proj", "p1a")
    run_p2 = phases in ("full", "p2")

    for _rep in range(repeats):
        if _rep and barrier:
            # Serialize repeats (benchmarking only) so the repetition
            # slope approximates single-shot latency.
            tc.strict_bb_all_engine_barrier()
        nc.vector.memset(sym_all, 0.0)

        # ---- Phase 1: xT (fp8), proj (fp8 DR), per-j rn + normalize ----
        with ExitStack() as ph1:
            tr_ps = ph1.enter_context(
                tc.tile_pool(name="tr_ps", bufs=2, space="PSUM"))
            proj_ps = ph1.enter_context(
                tc.tile_pool(name="proj_ps", bufs=2, space="PSUM"))
            n2_ps = ph1.enter_context(
                tc.tile_pool(name="n2_ps", bufs=2, space="PSUM"))
            bc_ps = ph1.enter_context(
                tc.tile_pool(name="bc_ps", bufs=2, space="PSUM"))

            for j in range(NB if run_p1 else 0):
                xt = xt_pool.tile([128, NK, SB], FP8, name="xt", tag="xt")
                for st in range(STPB):
                    s0 = j * SB + st * 128
                    xr = xraw_pool.tile([128, D], FP32, name="xr", tag="xr")
                    eng = nc.sync if st % 2 == 0 else nc.scalar
                    eng.dma_start(out=xr, in_=x_ap[s0:s0 + 128, :])
                    if p1_sub < 2:
                        continue
                    xb = xbf_pool.tile([128, D], BF16, name="xb", tag="xb")
                    nc.any.tensor_copy(out=xb, in_=xr)
                    tp = tr_ps.tile([128, NK, 128], BF16, name="tp", tag="tp")
                    for k in range(NK):
                        nc.tensor.transpose(
                            tp[:, k, :], xb[:, k * 128:(k + 1) * 128], ident)
                    nc.any.tensor_copy(
                        out=xt[:, :, st * 128:(st + 1) * 128], in_=tp)
                if p1_sub < 3:
                    continue

                n2 = n2_ps.tile([1, SB], FP32, name="n2", tag="n2")
                psb_j = psb_pool.tile([128, NK, SB], BF16, name="psb",
                                      tag="psb")
                for e in range(NK):
                    pp = proj_ps.tile([128, SB], FP32, name="pp", tag="pp")
                    for t in range(NP):
                        nc.tensor.matmul(
                            pp,
                            lhsT=w8[:, 2 * t:2 * t + 2,
                                    e * 128:(e + 1) * 128],
                            rhs=xt[:, 2 * t:2 * t + 2, :],
                            start=(t == 0),
                            stop=(t == NP - 1),
                            perf_mode=DR,
                        )
                    nc.any.tensor_copy(out=psb_j[:, e, :], in_=pp)
                    if p1_sub < 3.5:
                        continue
                    # square the evicted bf16 copy, not the PSUM tile: pp is
                    # then freed after a single read and the proj matmul
                    # stream never stalls on PSUM recycling.
                    sq = sq_pool.tile([128, SB], BF16, name="sq", tag="sq")
                    nc.any.tensor_mul(sq, psb_j[:, e, :], psb_j[:, e, :])
                    nc.tensor.matmul(n2, lhsT=ones_colb, rhs=sq,
                                     start=(e == 0), stop=(e == NK - 1))
                if p1_sub < 4:
                    continue

                # rn = 16 / max(sqrt(n2), eps) for this block, then
                # normed8[:, :, jSB] = psb_j * bc  (bc = broadcast 16*rn)
                y = small_pool.tile([1, SB], FP32, name="y", tag="sm")
                nc.scalar.activation(out=y, in_=n2, func=AF.Sqrt)
                nc.vector.tensor_scalar_max(y, y, EPS)
                rn_j = rn_r[:, j * SB:(j + 1) * SB]
                with nc.allow_low_precision(reason="fp32r has fp32 bits"):
                    nc.vector.reciprocal(rn_j, y)
                bc = bc_ps.tile([128, SB], FP32, name="bc", tag="bc")
                nc.tensor.matmul(bc, lhsT=ones_row, rhs=rn_j,
                                 start=True, stop=True)
                bcs = bcs_pool.tile([128, SB], BF16, name="bcs", tag="bcs")
                nc.any.tensor_copy(out=bcs, in_=bc)
                for e in range(NK):
                    nc.any.tensor_mul(
                        normed8[:, e, j * SB:(j + 1) * SB],
                        psb_j[:, e, :], bcs)

        # ---- Phase 2: sim (upper super-block triangle), sigmoid, sums ----
        with ExitStack() as ph2:
            attn_ps = ph2.enter_context(
                tc.tile_pool(name="attn_ps", bufs=2, space="PSUM"))
            cs_ps = ph2.enter_context(
                tc.tile_pool(name="cs_ps", bufs=1, space="PSUM"))
            tps_ps = ph2.enter_context(
                tc.tile_pool(name="tps_ps", bufs=1, space="PSUM"))

            G = 2                 # j-blocks per sigmoid batch group
            NG = NB // G
            accs = [acc_pool.tile([128, NG], FP32, name=f"acc{i}")
                    for i in range(NST)]

            for g in range(NG if run_p2 else 0):
                j0 = g * G            # group covers j in [j0, j0+G)
                n_i = min(NST, 4 * (j0 + G - 1) + 4)
                cs = {}
                for jj in range(G):
                    if min(NST, 4 * (j0 + jj)) > 0:
                        cs[jj] = cs_ps.tile([1, SB], FP32, name="cs",
                                            tag=f"cs{jj}")
                scr = None
                for i in range(n_i):
                    # j-blocks this i-tile needs within the group
                    jlo = max(0, (i - 3 + 3) // 4 - j0)  # first jj with i<4j+4
                    jlo = 0
                    while 4 * (j0 + jlo) + 4 <= i:
                        jlo += 1
                    apt = attn_ps.tile([128, G * SB], FP32, name="att",
                                       tag="att")
                    for t in range(NP):
                        for jj in range(jlo, G):
                            j = j0 + jj
                            nc.tensor.matmul(
                                apt[:, jj * SB:(jj + 1) * SB],
                                lhsT=normed8[:, 2 * t:2 * t + 2,
                                             i * 128:(i + 1) * 128],
                                rhs=normed8[:, 2 * t:2 * t + 2,
                                            j * SB:(j + 1) * SB],
                                start=(t == 0),
                                stop=(t == NP - 1),
                                perf_mode=DR,
                            )
                    if i % 2 == 0:
                        scr = scr_pool.tile([128, 2, G * SB], FP8,
                                            name="scr", tag="scr")
                    nc.scalar.activation(
                        out=scr[:, i % 2, jlo * SB:G * SB],
                        in_=apt[:, jlo * SB:G * SB],
                        func=AF.Sigmoid,
                        scale=1.0 / (FP8_SCALE * FP8_SCALE),
                        accum_out=accs[i][:, g:g + 1])
                    if i % 2 == 1:
                        for jj in range(G):
                            n_cs = min(NST, 4 * (j0 + jj))
                            if i < n_cs:
                                nc.tensor.matmul(
                                    cs[jj],
                                    lhsT=ones8,
                                    rhs=scr[:, :, jj * SB:(jj + 1) * SB],
                                    start=(i == 1),
                                    stop=(i == n_cs - 1),
                                    perf_mode=DR)
                for jj in sorted(cs):
                    j = j0 + jj
                    cs_sb = small_pool.tile([1, SB], FP32, name="cs_sb",
                                            tag="sm")
                    nc.vector.tensor_copy(out=cs_sb, in_=cs[jj])
                    for c in range(STPB):
                        tp2 = tps_ps.tile([128, 1], FP32, name="tp2",
                                          tag="tp2")
                        nc.tensor.transpose(
                            tp2, cs_sb[:, c * 128:(c + 1) * 128],
                            ident1)
                        nc.vector.tensor_copy(
                            out=sym_all[:, STPB * j + c: STPB * j + c + 1],
                            in_=tp2)

            if run_p2:
                for i in range(NST):
                    nc.vector.tensor_reduce(out_all[:, i:i + 1],
                                            accs[i][:, i // (STPB * G):NG],
                                            axis=AX.X, op=ALU.add)
                nc.vector.tensor_add(out_fin, out_all, sym_all)
                nc.vector.tensor_scalar_mul(out_fin, out_fin, 1.0 / s_total)
                nc.sync.dma_start(out=out_ap.rearrange("(i p) -> p i", p=128),
                                  in_=out_fin)
            else:
                nc.sync.dma_start(out=out_ap.rearrange("(i p) -> p i", p=128),
                                  in_=sym_all)


def build(s_total=S, repeats=1, phases="full", barrier=True,
          alloc_mode="stack"):
    nc = bacc.Bacc("TRN2", target_bir_lowering=False, debug=False)
    x_t = nc.dram_tensor("x", [s_total, D], FP32, kind="ExternalInput")
    w_t = nc.dram_tensor("w", [D, D], FP32, kind="ExternalInput")
    o_t = nc.dram_tensor("out", [s_total], FP32, kind="ExternalOutput")
    with tile.TileContext(nc, pool_alloc_mode=alloc_mode) as tc:
        with ExitStack() as ctx:
            emit(ctx, tc, o_t[:], x_t[:, :], w_t[:, :], s_total=s_total,
                 repeats=repeats, phases=phases, barrier=barrier)
    nc.compile()
    return nc


def _run(x, W, trace=False, **kwargs):
    from concourse.bass_utils import run_bass_kernel_spmd

    x = np.ascontiguousarray(np.asarray(x, dtype=np.float32))
    W = np.ascontiguousarray(np.asarray(W, dtype=np.float32))
    assert x.shape == (B, S, D) and W.shape == (D, D)
    nc = build()
    in_maps = [{"x": np.ascontiguousarray(x[b]), "w": W} for b in range(B)]
    res = run_bass_kernel_spmd(nc, in_maps, core_ids=list(range(N_CORES)),
                               trace=trace, **kwargs)
    out = np.stack([r["out"] for r in res.results])[:, None, :]
    return out.astype(np.float32), res


def kernel(x, W):
    out, _ = _run(x, W)
    return out
